# revision 1
# baseline (speedup 1.0000x reference)
"""Causal self-attention (B=8, T=1024, C=768, H=12) on 8 trn2 NeuronCores.

Data-parallel: one batch element per core, no collectives.  All matmul
tensors bf16 (measured 250,034 ns/iter vs 292,000 ns for f32r on the
in-NEFF hw-loop marginal harness; rel err 3.24e-3 vs the f32 reference,
well under the 2e-2 gate).  Per-matmul HW cost is ~N*0.42ns + ~130ns fixed
(microbenched), so the 682-matmul schedule is instruction-overhead bound;
bf16 wins via halved weight DMA + cheaper weight loads.
Changes vs v1:
  - Diagonal S^T blocks are computed at narrowed N (valid width, >=256 for
    f32r full rate) written at PSUM bank start with the rhs q-offset shifted;
    the exp AP un-shifts them into the q-aligned pt tile.  Saves ~1280
    PE cycles/head without violating the bank-aligned-output ISA rule.
  - P (post-exp attention weights) and V are bf16: halves DVE mask-multiply
    and Pool memset cost and PV SBUF traffic.  S/QKV/proj stay f32r.
  - Phase-1 DMA spread over three HWDGE queues (sync/vector/gpsimd) with
    w_v prefetched on the scalar queue, removing the x-load serialization.
  - out DMA merged to one descriptor per 128-row tile.
"""

import sys

if "/opt/trn_rl_repo" not in sys.path:
    sys.path.insert(0, "/opt/trn_rl_repo")

from contextlib import ExitStack

import numpy as np

import concourse.bass as bass
import concourse.bacc as bacc
import concourse.mybir as mybir
from concourse import tile
from concourse.masks import make_identity

P = 128
T = 1024
C = 768
H = 12
D = 64
TT = T // P          # 8 t-tiles
KC = C // P          # 6 c-tiles (contraction)
NQK = 2 * C // P     # 12 q/k M-tiles
VW = H * (D + 1)     # 780: v with interleaved ones columns

F32 = mybir.dt.float32
F32R = mybir.dt.float32r
BF16 = mybir.dt.bfloat16


def build_nc(mm_dt: str = "f32r", repeat: int = 1, hw_loop: int = 0):
    MDT = {"bf16": BF16, "f32r": F32R, "f32": F32}[mm_dt]  # qkv/proj matmul dtype
    PDT = BF16                                             # P/V attention dtype

    nc = bacc.Bacc(None)
    x_d = nc.declare_dram_parameter("x", [T, C], F32, isOutput=False)
    wa_d = nc.declare_dram_parameter("w_attn", [C, 3 * C], MDT, isOutput=False)
    ba_d = nc.declare_dram_parameter("b_attn", [3 * C], F32, isOutput=False)
    wp_d = nc.declare_dram_parameter("w_proj", [C, C], MDT, isOutput=False)
    bp_d = nc.declare_dram_parameter("b_proj", [C], F32, isOutput=False)
    out_d = nc.declare_dram_parameter("out", [T, C], F32, isOutput=True)

    with tile.TileContext(nc) as tc, ExitStack() as ctx:
        const = ctx.enter_context(tc.tile_pool(name="const", bufs=1))
        identity = const.tile([P, P], F32)
        make_identity(nc, identity)
        # 0/1 triangle mask for diagonal blocks: cm01[p,c] = 1 if c >= p else 0
        cm01 = const.tile([P, P], PDT)
        nc.gpsimd.memset(cm01[:], 1.0)
        nc.gpsimd.affine_select(
            out=cm01[:],
            in_=cm01[:],
            compare_op=mybir.AluOpType.is_ge,
            fill=0.0,
            base=0,
            pattern=[[1, P]],
            channel_multiplier=-1,
        )
        ba_cols = const.tile([P, NQK], F32)
        bav = const.tile([P, C], F32)
        bpb = const.tile([P, C], F32)

        persist = ctx.enter_context(tc.tile_pool(name="persist", bufs=1))
        xT = persist.tile([P, KC, T], MDT)      # x^T: [c%128, c//128, t]
        wv = persist.tile([P, KC, C], MDT)      # w_attn[:, 2C:3C]
        wp = persist.tile([P, KC, C], MDT)      # w_proj
        wa_all = persist.tile([P, KC, NQK * P], MDT)  # q/k weight tiles
        v_all = persist.tile([P, TT, VW], PDT)  # v + ones cols (bf16)
        yT = persist.tile([P, KC, T], MDT)      # y^T (normalized)
        # only the interleaved per-head ones-columns need the 1.0 fill; the
        # data columns are fully overwritten by the v evictions
        nc.gpsimd.memset(
            v_all[:].rearrange("p t (h e) -> p (t h) e", e=D + 1)[:, :, D : D + 1], 1.0
        )

        xpool = ctx.enter_context(tc.tile_pool(name="xpool", bufs=3))
        mm_psum = ctx.enter_context(tc.tile_pool(name="mm_psum", bufs=2, space="PSUM"))

        qkpool = ctx.enter_context(tc.tile_pool(name="qkpool", bufs=4))
        st_psum = ctx.enter_context(tc.tile_pool(name="st_psum", bufs=2, space="PSUM"))
        y_psum = ctx.enter_context(tc.tile_pool(name="y_psum", bufs=2, space="PSUM"))
        ptpool = ctx.enter_context(tc.tile_pool(name="ptpool", bufs=3))
        zpool = ctx.enter_context(tc.tile_pool(name="zpool", bufs=2))
        zrpool = ctx.enter_context(tc.tile_pool(name="zrpool", bufs=2))
        outpool = ctx.enter_context(tc.tile_pool(name="outpool", bufs=2))
        import contextlib

        loop_cm = (
            tc.For_i(
                0,
                hw_loop,
                1,
                hint_engines=(
                    mybir.EngineType.PE,
                    mybir.EngineType.DVE,
                    mybir.EngineType.Activation,
                    mybir.EngineType.SP,
                    mybir.EngineType.Pool,
                ),
            )
            if hw_loop
            else contextlib.nullcontext()
        )
        with loop_cm:
            for _rep in range(repeat):
                # warm the PE clock gate while the first x tiles are in flight
                warm_ps = mm_psum.tile([P, 512], F32, tag="mm", name="warm")
                for _ in range(10):
                    nc.tensor.transpose(warm_ps[:, :P], identity[:], identity[:])

                # ---- phase 1: transpose x, compute v ----
                # x tiles split over the sync HWDGE queue (even) and the gpsimd
                # SWDGE queue (odd, Pool is idle here); w_v + biases behind the
                # first x tiles on the scalar HWDGE queue.  The shared DMA fabric
                # round-robins the queues, so x is never head-of-line blocked by
                # the 2.4MB w_v transfer.
                xts = {}
                xq = [nc.sync, nc.gpsimd]

                def load_x(tt):
                    xt = xpool.tile([P, C], F32, tag="x", name="xt")
                    xq[tt % 2].dma_start(xt[:], x_d[tt * P : (tt + 1) * P, :])
                    xts[tt] = xt

                for tt in range(TT):
                    load_x(tt)
                wa_v = wa_d[:, 2 * C : 3 * C].rearrange("(a p) n -> p a n", p=P)
                nc.scalar.dma_start(wv[:, :, :384], wa_v[:, :, :384])
                nc.scalar.dma_start(bav[:], ba_d[2 * C : 3 * C][None, :].to_broadcast((P, C)))
                nc.scalar.dma_start(wv[:, :, 384:], wa_v[:, :, 384:])
                nc.scalar.dma_start(
                    ba_cols[:], ba_d[: 2 * C].rearrange("(a p) -> p a", p=P)
                )
                nc.scalar.dma_start(bpb[:], bp_d[:][None, :].to_broadcast((P, C)))

                def trans_x(tt):
                    xt = xts.pop(tt)
                    for grp in range(2):
                        pst = mm_psum.tile([P, 512], F32, tag="mm", name="tps")
                        for j in range(3):
                            kc = 3 * grp + j
                            nc.tensor.transpose(
                                pst[:, j * P : (j + 1) * P], xt[:, kc * P : (kc + 1) * P], identity
                            )
                        nc.vector.tensor_copy(
                            xT[:, 3 * grp : 3 * grp + 3, tt * P : (tt + 1) * P],
                            pst[:, :384].rearrange("p (a b) -> p a b", b=P),
                        )

                def v_mm(tt, nn):
                    pst = mm_psum.tile([P, 512], F32, tag="mm", name="vps")
                    ps = pst[:, :384]
                    for kc in range(KC):
                        nc.tensor.matmul(
                            ps,
                            xT[:, kc, tt * P : (tt + 1) * P],
                            wv[:, kc, nn * 384 : (nn + 1) * 384],
                            start=(kc == 0),
                            stop=(kc == KC - 1),
                        )
                    vview = v_all[:, tt, :].rearrange("p (h e) -> p h e", e=D + 1)[
                        :, nn * 6 : (nn + 1) * 6, :D
                    ]
                    nc.vector.tensor_add(
                        vview,
                        ps.rearrange("p (h e) -> p h e", e=D),
                        bav[:, nn * 384 : (nn + 1) * 384].rearrange("p (h e) -> p h e", e=D),
                    )

                # transposes chase the arriving x tiles; v matmuls backfill the
                # PE while later tiles are still in flight
                trans_x(0)
                trans_x(1)
                for tt in range(2, TT):
                    trans_x(tt)
                    v_mm(tt - 2, 0)
                    v_mm(tt - 2, 1)
                for tt in range(TT - 2, TT):
                    v_mm(tt, 0)
                    v_mm(tt, 1)

                # ---- phase 2: q^T/k^T M-tile pairs + attention per head ----

                # prefetch all q/k weight tiles (sync queue, overlapped with
                # attention compute) and w_proj (scalar queue)
                wa_r = wa_d[:, :].rearrange("(a p) n -> p a n", p=P)
                for m in (0, 6, 1, 7, 2, 8, 3, 9, 4, 10, 5, 11):
                    nc.sync.dma_start(
                        wa_all[:, :, m * P : (m + 1) * P], wa_r[:, :, m * P : (m + 1) * P]
                    )
                nc.scalar.dma_start(wp[:], wp_d[:, :].rearrange("(a p) n -> p a n", p=P))

                qk_t = {}

                def emit_qk(m):
                    qt = qkpool.tile([P, T], MDT, tag="qk", name="qt")
                    qk_t[m] = qt
                    for nn in range(2):
                        ps = mm_psum.tile([P, 512], F32, tag="mm", name="qps")
                        for kc in range(KC):
                            nc.tensor.matmul(
                                ps,
                                wa_all[:, kc, m * P : (m + 1) * P],
                                xT[:, kc, nn * 512 : (nn + 1) * 512],
                                start=(kc == 0),
                                stop=(kc == KC - 1),
                            )
                        nc.vector.tensor_scalar_add(
                            qt[:, nn * 512 : (nn + 1) * 512], ps, ba_cols[:, m : m + 1]
                        )

                emit_qk(0)
                emit_qk(6)
                for pr in range(6):
                    # Head pair: head A (even) at qk-tile partitions 0-63, head B
                    # (odd) at 64-127.  The two S^T matmuls per k-tile write the
                    # two halves of one [128,1024] PSUM tile; one exp covers both.
                    hA, hB = 2 * pr, 2 * pr + 1
                    for J in range(2):
                        nk = 4 * J + 4
                        ypA = y_psum.tile([D + 1, 512], F32, tag="y", name="ypA")
                        ypB = y_psum.tile([D + 1, 512], F32, tag="y", name="ypB")
                        pts = {}
                        for i in range(nk):
                            jj = i - 4 * J  # >=1: leading 128*jj q-cols fully masked
                            st = st_psum.tile([P, 1024], F32, tag="st")
                            pt = ptpool.tile([P, 1024], PDT, tag="pt")
                            pts[i] = pt
                            if i >= 4 * J and jj > 0:
                                # narrowed diag block: compute only the valid tail
                                # (>=256 wide for f32r full rate) at bank start
                                w_v = 512 - P * jj          # valid width
                                w_pad = max(w_v, 256)
                                off = 512 - w_pad           # rhs q-offset in window
                                for s, hh in ((0, hA), (1, hB)):
                                    par = D * (hh % 2)
                                    nc.tensor.matmul(
                                        st[:, s * 512 : s * 512 + w_pad],
                                        qk_t[6 + pr][par : par + D, i * P : (i + 1) * P],
                                        qk_t[pr][
                                            par : par + D,
                                            J * 512 + off : (J + 1) * 512,
                                        ],
                                        start=True,
                                        stop=True,
                                    )
                                # exp the valid tail into the q-aligned pt columns
                                w0 = P * jj
                                st2 = st[:, :].rearrange("p (s c) -> p s c", s=2)
                                pt2 = pt[:, :].rearrange("p (s c) -> p s c", s=2)
                                nc.scalar.activation(
                                    pt2[:, :, w0:],
                                    st2[:, :, w_pad - w_v : w_pad],
                                    mybir.ActivationFunctionType.Exp,
                                    scale=0.125,
                                )
                                nc.gpsimd.memset(pt2[:, :, :w0], 0.0)
                            else:
                                for s, hh in ((0, hA), (1, hB)):
                                    par = D * (hh % 2)
                                    nc.tensor.matmul(
                                        st[:, s * 512 : (s + 1) * 512],
                                        qk_t[6 + pr][par : par + D, i * P : (i + 1) * P],
                                        qk_t[pr][par : par + D, J * 512 : (J + 1) * 512],
                                        start=True,
                                        stop=True,
                                    )
                                nc.scalar.activation(
                                    pt[:],
                                    st[:],
                                    mybir.ActivationFunctionType.Exp,
                                    scale=0.125,
                                )
                            if i >= 4 * J:
                                # zero the upper-triangular part of the diagonal block
                                blk = pt[:, :].rearrange("p (s c) -> p s c", s=2)[
                                    :, :, P * jj : P * (jj + 1)
                                ]
                                nc.vector.tensor_mul(
                                    blk,
                                    blk,
                                    cm01[:, None, :].to_broadcast((P, 2, P)),
                                )
                            # software pipeline: PV for k-tile i-1 lands after S^T(i)
                            todo = ([] if i == 0 else [i - 1]) + ([i] if i == nk - 1 else [])
                            for ip in todo:
                                ptp = pts.pop(ip)
                                for s, hh, yp in ((0, hA, ypA), (1, hB, ypB)):
                                    nc.tensor.matmul(
                                        yp[:, :],
                                        v_all[:, ip, (D + 1) * hh : (D + 1) * (hh + 1)],
                                        ptp[:, s * 512 : (s + 1) * 512],
                                        start=(ip == 0),
                                        stop=(ip == nk - 1),
                                    )
                        # interleave next qk-tile production: its matmuls fill the
                        # PE while this J-block's y evictions drain
                        if pr < 5:
                            emit_qk(pr + 1 if J == 0 else 7 + pr)
                        for hh, yp in ((hA, ypA), (hB, ypB)):
                            zr = zpool.tile([1, 512], F32, tag="z")
                            nc.vector.reciprocal(zr[0:1, :], yp[D : D + 1, :])
                            zrep = zrpool.tile([D, 512], F32, tag="zr")
                            nc.gpsimd.partition_broadcast(zrep[:], zr[0:1, :])
                            kc_y = hh // 2
                            par_y = D * (hh % 2)
                            nc.vector.tensor_mul(
                                yT[par_y : par_y + D, kc_y, J * 512 : (J + 1) * 512],
                                yp[:D, :],
                                zrep[:],
                            )

                # ---- phase 3: output projection ----
                for tt in range(TT):
                    ot = outpool.tile([P, C], F32, tag="out")
                    for nn in range(2):
                        pst = mm_psum.tile([P, 512], F32, tag="mm", name="pps")
                        ps = pst[:, :384]
                        for kc in range(KC):
                            nc.tensor.matmul(
                                ps,
                                yT[:, kc, tt * P : (tt + 1) * P],
                                wp[:, kc, nn * 384 : (nn + 1) * 384],
                                start=(kc == 0),
                                stop=(kc == KC - 1),
                            )
                        nc.vector.tensor_add(
                            ot[:, nn * 384 : (nn + 1) * 384], ps, bpb[:, nn * 384 : (nn + 1) * 384]
                        )
                    nc.sync.dma_start(out_d[tt * P : (tt + 1) * P, :], ot[:])

    nc.finalize()
    return nc


_cache = {}
MM_DT = "bf16"


def get_nc():
    if "nc" not in _cache:
        _cache["nc"] = build_nc(mm_dt=MM_DT)
    return _cache["nc"]


def kernel(x, w_attn, b_attn, w_proj, b_proj):
    import ml_dtypes

    wdt = ml_dtypes.bfloat16 if MM_DT == "bf16" else np.float32
    x = np.ascontiguousarray(np.asarray(x, dtype=np.float32))
    w_attn = np.ascontiguousarray(np.asarray(w_attn, dtype=np.float32).astype(wdt))
    b_attn = np.ascontiguousarray(np.asarray(b_attn, dtype=np.float32))
    w_proj = np.ascontiguousarray(np.asarray(w_proj, dtype=np.float32).astype(wdt))
    b_proj = np.ascontiguousarray(np.asarray(b_proj, dtype=np.float32))

    from concourse.bass_utils import run_bass_kernel_spmd

    nc = get_nc()
    B = x.shape[0]
    assert B == 8
    in_maps = [
        dict(
            x=np.ascontiguousarray(x[b]),
            w_attn=w_attn,
            b_attn=b_attn,
            w_proj=w_proj,
            b_proj=b_proj,
        )
        for b in range(B)
    ]
    res = run_bass_kernel_spmd(nc, in_maps, list(range(B))).results
    return np.stack([res[b]["out"] for b in range(B)], axis=0)


if __name__ == "__main__":
    x = np.random.randn(8, T, C).astype(np.float32)
    w_attn = (np.random.randn(C, 3 * C) * 0.02).astype(np.float32)
    b_attn = np.zeros(3 * C, np.float32)
    w_proj = (np.random.randn(C, C) * 0.02).astype(np.float32)
    b_proj = np.zeros(C, np.float32)
    y = kernel(x, w_attn, b_attn, w_proj, b_proj)
    print(y.shape, y.dtype)

